# revision 6
# baseline (speedup 1.0000x reference)
"""Causal self-attention (B=2, T=2048, C=768, H=12) on 8 TRN2 NeuronCores.

Sharding: core i handles batch b = i//4 and 3 consecutive heads h0 = 3*(i%4).
Each core produces a partial projection output [T, C] (sum over its 3 heads);
the host sums the 4 partials per batch and adds biases.

Per-core dataflow (all transposeless):
  - QK gen:  psum[128,512] = sum_ct Wqk[ct,h].T @ xT[ct]  -> rows 0:64 = Q^T
             (scale+bias folded), rows 64:128 = K^T.
  - V gen:   psum[128,192] = sum_ct xT[ct,tchunk].T @ Wv[ct] -> v in natural
             [T, hs] layout, stored per k-tile as [v | 1] (ones col -> denom).
  - Attn:    S^T tile = K_block @ Q^T  ([128 kpos, 512 q] in PSUM), exp on ACT
             (no max subtraction; scores are O(1)), causal mask multiply on
             diagonal tiles only, PV accumulates [y^T | denom] over k-tiles.
  - Norm:    recip(denom) = exp(-ln d) on ACT, broadcast across partitions via
             a K=1 matmul, y^T = y_unnorm^T * bcast (read from PSUM).
  - Proj:    out[tchunk, :] = sum_h yT[h, tchunk].T @ Wp[h]  (PSUM -> DRAM,
             bf16 partials; host sums in f32).

Schedule: x streams in 512-token column chunks; QKV gen, normalize,
projection and output DMA are sprinkled between the attention rounds
(jq-major unit order) so every phase overlaps and the PE never sees a
long serial tail.
"""

import os

os.environ.setdefault("MYCRO_LOCAL_CACHE", "1")

import numpy as np
import ml_dtypes

BF16_NP = ml_dtypes.bfloat16

import concourse.bass as bass
import concourse.bacc as bacc
import concourse.mybir as mybir
import concourse.tile as tile
from concourse.bass_utils import run_bass_kernel_spmd

T = 2048
C = 768
HS = 64
NH = 12
HPC = 3  # heads per core
NCORES = 8
CT = C // 128  # 6 contraction tiles for qkv/v gen
QC = 512  # q-chunk width
NQC = T // QC  # 4
NKT = T // 128  # 16 k-tiles (and T-chunks)
SCALE = 1.0 / 8.0  # 1/sqrt(HS)
F32 = mybir.dt.float32
BF16 = mybir.dt.bfloat16

_PROGRAM = None
DEBUG_DUMP = False


class _Bacc(bacc.Bacc):
    # Pin the ACT function-table set: Exp and Ln both live in set 6
    # (natural_log_exp_and_others, canonical act_info.json index), so one
    # table load at the top covers every activation with no mid-kernel
    # switches. The stock greedy inserter toggles exp_and_others <->
    # natural_log around each Ln (1.3us per reload).
    def insert_act_table_loads(self):
        for b in self.main_func.blocks:
            idx = None
            for i, ins in enumerate(b.instructions):
                if isinstance(ins, mybir.InstActivation):
                    idx = i
                    break
            if idx is None:
                continue
            load = mybir.InstLoadActFuncSet(
                name=self.get_next_instruction_name(),
                ins=[],
                outs=[],
                act_func_set_id=6,
            )
            load.engine = mybir.EngineType.Activation
            insts = list(b.instructions)
            insts.insert(idx, load)
            b.instructions = insts


def _build_program():
    nc = _Bacc("TRN2")
    xT_d = nc.declare_dram_parameter("xT", [128, NQC, CT, QC], BF16, isOutput=False)
    wqk_d = nc.declare_dram_parameter("wqk", [128, CT, HPC, 128], BF16, isOutput=False)
    wv_d = nc.declare_dram_parameter("wv", [128, CT, HPC * HS], BF16, isOutput=False)
    wp_d = nc.declare_dram_parameter("wp", [128, HPC, C], BF16, isOutput=False)
    bq_d = nc.declare_dram_parameter("bq", [HS, HPC], F32, isOutput=False)
    mask_d = nc.declare_dram_parameter("mask", [128, QC], BF16, isOutput=False)
    out_d = nc.declare_dram_parameter("out", [T, C], BF16, isOutput=True)
    dbg = {}
    if DEBUG_DUMP:
        for _n in ("dbg_q", "dbg_k", "dbg_yun", "dbg_y", "dbg_rcp"):
            dbg[_n] = nc.declare_dram_parameter(_n, [128, HPC, T], F32, isOutput=True)
        dbg["dbg_v"] = nc.declare_dram_parameter(
            "dbg_v", [128, NKT, HPC, HS + 1], F32, isOutput=True
        )

    with tile.TileContext(nc) as tc:
        with (
            tc.tile_pool(name="const", bufs=1) as constp,
            tc.tile_pool(name="big", bufs=1) as bigp,
            tc.tile_pool(name="exps", bufs=8) as expp,
            tc.tile_pool(name="work", bufs=3) as workp,
            tc.tile_pool(name="ps_s", bufs=1, space="PSUM") as ps_s,
            tc.tile_pool(name="ps_y", bufs=1, space="PSUM") as ps_y,
            tc.tile_pool(name="ps_m", bufs=2, space="PSUM") as ps_m,
        ):
            wqk = constp.tile([128, CT, HPC, 128], BF16)
            wv = constp.tile([128, CT, HPC * HS], BF16)
            wp = constp.tile([128, HPC, C], BF16)
            bq = constp.tile([HS, HPC], F32)
            mask = constp.tile([128, QC], BF16)
            ones = constp.tile([128, HS], BF16)
            scr = constp.tile([128, QC], BF16)
            xsb = bigp.tile([128, NQC, CT, QC], BF16)
            qT = bigp.tile([128, HPC, T], BF16)
            kT = bigp.tile([128, HPC, T], BF16)
            vsb = bigp.tile([128, NKT, HPC, HS + 1], BF16)
            yun = bigp.tile([128, HPC, T], BF16)  # rows 0:64 y, row 64 denom
            rcp = bigp.tile([128, HPC, T], BF16)  # row 64 = 1/denom
            yT = bigp.tile([128, HPC, T], BF16)

            # ---- lead-in work that needs no DMA: memsets, ACT table prime,
            # and HAM warmup matmuls so the first real matmul runs at 2.4 GHz
            nc.vector.memset(ones, 1.0)
            nc.vector.memset(scr, 1.0)
            dact = workp.tile([128, 8], F32, tag="ob", name="dact")
            nc.scalar.activation(
                dact[0:1, 0:8], scr[0:1, 0:8], mybir.ActivationFunctionType.Exp
            )
            # zero the K-padding halves: K=64 contractions are padded to K=128
            # (zero rows are numerically free; keeps the HAM clock governor at
            # 2.4 GHz since half-array row-group matmuls don't count as busy)
            nc.vector.memset(qT[HS:128, :, :], 0.0)
            nc.gpsimd.memset(kT[HS:128, :, :], 0.0)
            nc.gpsimd.memset(yT[HS:128, :, :], 0.0)
            nc.gpsimd.memset(vsb[:, :, :, HS], 1.0)
            for i in range(8):
                wps = ps_m.tile([128, QC], F32, tag="misc", name=f"warm{i}")
                nc.tensor.matmul(wps, scr[:, 0:128], scr, start=True, stop=True)

            # ---- input DMAs, priority order (first matmul needs wqk + x
            # chunk 0 only; wp is not needed until the first projection)
            nc.sync.dma_start(out=wqk, in_=wqk_d[:])
            nc.sync.dma_start(out=xsb[:, 0], in_=xT_d[:, 0])
            nc.sync.dma_start(out=wv, in_=wv_d[:])
            nc.sync.dma_start(out=bq, in_=bq_d[:])
            nc.sync.dma_start(out=mask, in_=mask_d[:])
            for cchunk in (1, 2, 3):
                nc.sync.dma_start(out=xsb[:, cchunk], in_=xT_d[:, cchunk])
            nc.sync.dma_start(out=wp, in_=wp_d[:])

            # ---- QKV generation units (sprinkled into the attention stream)
            def qk_unit(h, jq):
                pqk = ps_m.tile([128, QC], F32, tag="misc", name=f"pqk{h}_{jq}")
                for ct in range(CT):
                    nc.tensor.matmul(
                        pqk,
                        wqk[:, ct, h, :],
                        xsb[:, jq, ct, :],
                        start=(ct == 0),
                        stop=(ct == CT - 1),
                    )
                nc.vector.tensor_scalar_add(
                    qT[0:HS, h, jq * QC : (jq + 1) * QC],
                    pqk[0:HS, :],
                    bq[:, h : h + 1],
                )
                # partition-shifting evacuation (64:128 -> 0:64)
                nc.vector.tensor_copy(
                    kT[0:HS, h, jq * QC : (jq + 1) * QC], pqk[64:128, :]
                )

            def v_unit(m):
                pv = ps_m.tile([128, QC], F32, tag="misc", name=f"pv{m}")
                jq, sub = m // 4, m % 4
                for ct in range(CT):
                    nc.tensor.matmul(
                        pv[:, 0 : HPC * HS],
                        xsb[:, jq, ct, sub * 128 : (sub + 1) * 128],
                        wv[:, ct, :],
                        start=(ct == 0),
                        stop=(ct == CT - 1),
                    )
                nc.vector.tensor_copy(
                    vsb[:, m, :, 0:HS],
                    pv[:, 0 : HPC * HS].rearrange("p (h d) -> p h d", h=HPC),
                )

            # ---- attention machinery
            def tile_geom(jq, kt):
                if kt < 4 * jq:  # full k-tile
                    return QC, 0
                r = kt - 4 * jq
                return QC - 128 * r, 128 * r

            def s_mms(es_p, jq, h, g):
                q0 = jq * QC
                for s in range(2):
                    kt = 2 * g + s
                    w, qoff = tile_geom(jq, kt)
                    nc.tensor.matmul(
                        es_p[:, s * QC : s * QC + w],
                        kT[:, h, kt * 128 : (kt + 1) * 128],
                        qT[:, h, q0 + qoff : q0 + QC],
                        start=True,
                        stop=True,
                    )

            def exp_mask(es_p, es_b, jq, g):
                kt0 = 2 * g
                if kt0 + 1 < 4 * jq:  # both full
                    nc.scalar.activation(
                        es_b[:, 0 : 2 * QC],
                        es_p[:, 0 : 2 * QC],
                        mybir.ActivationFunctionType.Exp,
                    )
                else:
                    r0 = kt0 - 4 * jq  # 0 or 2
                    if r0 == 0:  # widths 512, 384: one contiguous span
                        nc.scalar.activation(
                            es_b[:, 0 : QC + 384],
                            es_p[:, 0 : QC + 384],
                            mybir.ActivationFunctionType.Exp,
                        )
                    else:  # widths 256, 128: two disjoint spans
                        nc.scalar.activation(
                            es_b[:, 0:256],
                            es_p[:, 0:256],
                            mybir.ActivationFunctionType.Exp,
                        )
                        nc.scalar.activation(
                            es_b[:, QC : QC + 128],
                            es_p[:, QC : QC + 128],
                            mybir.ActivationFunctionType.Exp,
                        )
                    for s in range(2):
                        w = QC - 128 * (kt0 + s - 4 * jq)
                        nc.vector.tensor_mul(
                            es_b[:, s * QC : s * QC + w],
                            es_b[:, s * QC : s * QC + w],
                            mask[:, 0:w],
                        )

            def pv_mms(py, es_b, jq, h, g):
                for s in range(2):
                    kt = 2 * g + s
                    w, qoff = tile_geom(jq, kt)
                    nc.tensor.matmul(
                        py[0 : HS + 1, qoff:QC],
                        vsb[:, kt, h, :],
                        es_b[:, s * QC : s * QC + w],
                        start=(kt == 0),
                        stop=(kt == 4 * jq + 3),
                        skip_group_check=True,
                    )

            def recip(jq):
                # 1/d = exp(-ln d) on ACT over this q-chunk, all heads
                c0, c1 = jq * QC, (jq + 1) * QC
                nc.scalar.activation(
                    rcp[64:65, :, c0:c1],
                    yun[64:65, :, c0:c1],
                    mybir.ActivationFunctionType.Ln,
                )
                nc.scalar.activation(
                    rcp[64:65, :, c0:c1],
                    rcp[64:65, :, c0:c1],
                    mybir.ActivationFunctionType.Exp,
                    scale=-1.0,
                )

            def normalize(jq, h):
                q0 = jq * QC
                pb = ps_m.tile([128, QC], F32, tag="misc", name=f"pb{jq}_{h}")
                nc.tensor.matmul(
                    pb[0:HS, :],
                    ones[64:65, 0:HS],
                    rcp[64:65, h, q0 : q0 + QC],
                    start=True,
                    stop=True,
                )
                nc.vector.tensor_mul(
                    yT[0:HS, h, q0 : q0 + QC],
                    yun[0:HS, h, q0 : q0 + QC],
                    pb[0:HS, :],
                )

            def proj(t):
                ob = workp.tile([128, C], BF16, tag="ob", name=f"ob{t}")
                for n0, w in ((0, 512), (512, 256)):
                    po = ps_m.tile([128, QC], F32, tag="misc", name=f"po{t}_{n0}")
                    for h in range(HPC):
                        nc.tensor.matmul(
                            po[:, 0:w],
                            yT[:, h, t * 128 : (t + 1) * 128],
                            wp[:, h, n0 : n0 + w],
                            start=(h == 0),
                            stop=(h == HPC - 1),
                        )
                    nc.vector.tensor_copy(ob[:, n0 : n0 + w], po[:, 0:w])
                nc.sync.dma_start(out=out_d[t * 128 : (t + 1) * 128, :], in_=ob)

            # ---- pair driver: two independent (jq, h) units interleave so
            # the PE streams one unit's matmuls while ACT runs the other's
            # exp; `extras` (QKV gen / normalize / proj closures) are emitted
            # between the S and PV matmuls of each round to fill PE stalls.
            def run_pair(pi, units2, extras):
                lanes = []
                for li, (jq, h) in enumerate(units2):
                    lanes.append(
                        {
                            "jq": jq,
                            "h": h,
                            "G": 2 * jq + 2,
                            "py": ps_y.tile(
                                [128, QC], F32, tag=f"py{li}", name=f"py{li}_{pi}"
                            ),
                            "li": li,
                            "ebs": {},
                        }
                    )
                rounds = max(ln["G"] for ln in lanes) + 1
                n = len(extras)
                # extra i fires in round floor(i * rounds / n)
                sched = [[] for _ in range(rounds)]
                for i, ex in enumerate(extras):
                    sched[i * rounds // n].append(ex)
                for g in range(rounds):
                    for ln in lanes:
                        if g < ln["G"]:
                            es_p = ps_s.tile(
                                [128, 2 * QC],
                                F32,
                                tag=f"es{ln['li']}",
                                name=f"es{ln['li']}_{pi}_{g}",
                            )
                            es_b = expp.tile([128, 2 * QC], BF16, tag="ex")
                            ln["ebs"][g] = es_b
                            s_mms(es_p, ln["jq"], ln["h"], g)
                            exp_mask(es_p, es_b, ln["jq"], g)
                    for ex in sched[g]:
                        ex()
                    for ln in lanes:
                        if 0 <= g - 1 < ln["G"]:
                            pv_mms(
                                ln["py"],
                                ln["ebs"].pop(g - 1),
                                ln["jq"],
                                ln["h"],
                                g - 1,
                            )
                        if g - 1 == ln["G"] - 1:
                            # stash unnormalized y + denominator row (bf16);
                            # frees the PSUM bank for the next pair
                            jq, h = ln["jq"], ln["h"]
                            nc.vector.tensor_copy(
                                yun[0 : HS + 1, h, jq * QC : (jq + 1) * QC],
                                ln["py"][0 : HS + 1, :],
                            )

            QK = qk_unit
            VG = v_unit

            # preamble: head 0/1 q-chunk 0 so pair 0 can start immediately
            QK(0, 0)
            QK(1, 0)

            run_pair(
                0,
                [(0, 0), (0, 1)],
                [
                    lambda: QK(2, 0),
                    lambda: VG(0),
                    lambda: VG(1),
                    lambda: VG(2),
                    lambda: VG(3),
                    lambda: QK(0, 1),
                    lambda: QK(1, 1),
                ],
            )
            run_pair(
                1,
                [(0, 2), (1, 0)],
                [
                    lambda: QK(2, 1),
                    lambda: VG(4),
                    lambda: VG(5),
                    lambda: VG(6),
                    lambda: VG(7),
                    lambda: QK(0, 2),
                    lambda: QK(1, 2),
                ],
            )
            run_pair(
                2,
                [(1, 1), (1, 2)],
                [
                    lambda: recip(0),
                    lambda: normalize(0, 0),
                    lambda: normalize(0, 1),
                    lambda: normalize(0, 2),
                    lambda: proj(0),
                    lambda: proj(1),
                    lambda: proj(2),
                    lambda: proj(3),
                    lambda: QK(2, 2),
                    lambda: VG(8),
                    lambda: VG(9),
                ],
            )
            run_pair(
                3,
                [(2, 0), (2, 1)],
                [
                    lambda: recip(1),
                    lambda: QK(0, 3),
                    lambda: normalize(1, 0),
                    lambda: normalize(1, 1),
                    lambda: normalize(1, 2),
                    lambda: proj(4),
                    lambda: proj(5),
                    lambda: proj(6),
                    lambda: proj(7),
                    lambda: VG(10),
                    lambda: VG(11),
                    lambda: VG(12),
                ],
            )
            run_pair(
                4,
                [(2, 2), (3, 0)],
                [
                    lambda: QK(1, 3),
                    lambda: QK(2, 3),
                    lambda: VG(13),
                    lambda: VG(14),
                    lambda: VG(15),
                ],
            )
            run_pair(
                5,
                [(3, 1), (3, 2)],
                [
                    lambda: recip(2),
                    lambda: normalize(2, 0),
                    lambda: normalize(2, 1),
                    lambda: normalize(2, 2),
                    lambda: proj(8),
                    lambda: proj(9),
                    lambda: proj(10),
                    lambda: proj(11),
                ],
            )
            # tail: last q-chunk only
            recip(3)
            for h in range(HPC):
                normalize(3, h)
            for t in range(12, NKT):
                proj(t)
            if DEBUG_DUMP:
                for name, tl in (
                    ("dbg_q", qT),
                    ("dbg_k", kT),
                    ("dbg_yun", yun),
                    ("dbg_y", yT),
                    ("dbg_rcp", rcp),
                ):
                    st = workp.tile([128, HPC, T], F32, tag="dbgst")
                    nc.vector.tensor_copy(st, tl)
                    nc.sync.dma_start(out=dbg[name][:], in_=st)
                stv = workp.tile([128, NKT, HPC, HS + 1], F32, tag="dbgst")
                nc.vector.tensor_copy(stv, vsb)
                nc.sync.dma_start(out=dbg["dbg_v"][:], in_=stv)
    return nc


def get_program():
    global _PROGRAM
    if _PROGRAM is None:
        _PROGRAM = _build_program()
        if not _PROGRAM.is_finalized():
            _PROGRAM.finalize()
    return _PROGRAM


def make_in_maps(x, W_attn, b_attn):
    x = np.asarray(x, dtype=np.float32)
    W_attn = np.asarray(W_attn, dtype=np.float32)
    b_attn = np.asarray(b_attn, dtype=np.float32)
    mask_arr = (
        np.arange(128, dtype=np.int64)[:, None] <= np.arange(QC, dtype=np.int64)[None, :]
    ).astype(BF16_NP)
    in_maps = []
    for i in range(NCORES):
        b = i // 4
        h0 = HPC * (i % 4)
        xb = x[b]  # [T, C]
        # chunk-major: [128, jq, ct, 512] so each q-chunk is one contiguous DMA
        xT_arr = np.ascontiguousarray(
            xb.T.reshape(CT, 128, NQC, QC).transpose(1, 2, 0, 3)
        ).astype(BF16_NP)
        Wq = (
            W_attn[:, h0 * HS : (h0 + HPC) * HS].reshape(C, HPC, HS) * SCALE
        )
        Wk = W_attn[:, C + h0 * HS : C + (h0 + HPC) * HS].reshape(C, HPC, HS)
        wqk_full = np.concatenate([Wq, Wk], axis=2)  # [C, HPC, 128]
        wqk_arr = np.ascontiguousarray(
            wqk_full.reshape(CT, 128, HPC, 128).transpose(1, 0, 2, 3)
        ).astype(BF16_NP)
        wv_arr = np.ascontiguousarray(
            W_attn[:, 2 * C + h0 * HS : 2 * C + (h0 + HPC) * HS]
            .reshape(CT, 128, HPC * HS)
            .transpose(1, 0, 2)
        ).astype(BF16_NP)
        bq_arr = np.ascontiguousarray(
            (b_attn[h0 * HS : (h0 + HPC) * HS] * SCALE).reshape(HPC, HS).T
        )
        in_maps.append(
            {
                "xT": xT_arr,
                "wqk": wqk_arr,
                "wv": wv_arr,
                "bq": bq_arr,
                "mask": mask_arr,
            }
        )
    return in_maps


def add_wp(in_maps, W_proj):
    W_proj = np.asarray(W_proj, dtype=np.float32)
    for i in range(NCORES):
        h0 = HPC * (i % 4)
        wp_arr = np.zeros((128, HPC, C), dtype=BF16_NP)
        wp_arr[:HS] = (
            W_proj[h0 * HS : (h0 + HPC) * HS, :]
            .reshape(HPC, HS, C)
            .transpose(1, 0, 2)
            .astype(BF16_NP)
        )
        in_maps[i]["wp"] = wp_arr
    return in_maps


def gather(results, b_attn, W_proj, b_proj):
    b_attn = np.asarray(b_attn, dtype=np.float32)
    W_proj = np.asarray(W_proj, dtype=np.float32)
    b_proj = np.asarray(b_proj, dtype=np.float32)
    parts = [np.asarray(r["out"], dtype=np.float32) for r in results]
    out = np.stack(
        [parts[0] + parts[1] + parts[2] + parts[3], parts[4] + parts[5] + parts[6] + parts[7]]
    )
    # b_v adds to y after normalization -> constant vector through the proj.
    # b_k provably cancels in softmax; b_q is handled on-device.
    const = b_proj + b_attn[2 * C : 3 * C] @ W_proj
    return out + const[None, None, :]


def run(x, W_attn, b_attn, W_proj, b_proj, trace=False):
    nc = get_program()
    in_maps = add_wp(make_in_maps(x, W_attn, b_attn), W_proj)
    res = run_bass_kernel_spmd(nc, in_maps, list(range(NCORES)), trace=trace)
    out = gather(res.results, b_attn, W_proj, b_proj)
    return out, res


def kernel(x, W_attn, b_attn, W_proj, b_proj):
    out, _ = run(x, W_attn, b_attn, W_proj, b_proj, trace=False)
    return out
